# revision 3
# baseline (speedup 1.0000x reference)
"""Trainium2 Bass kernel for nn_Attention_42202348650800 (v2).

Full causal attention block: fused QKV projection + RoPE + causal softmax
attention + output projection.  B=2, T=2048, HIDDEN=1024, H=HKV=16, D=64.

Sharding (8 NeuronCores): data-parallel over batch (2) x tensor-parallel over
heads (4 groups of 4 heads).  core c -> batch b = c // 4, head group g = c % 4
(heads 4g..4g+3).  Each core computes a partial output projection
y_partial^T = w_o[:, jslice].T @ ctx^T in [1024, 2048] (fp16); the host sums
the 4 partials of each batch.

v2 design notes (vs the first working kernel):
 - Single fine-grained schedule: QKV-projection / V-projection / output-
   projection matmuls are interleaved into the attention score/exp/ctx stream
   via a filler queue, so the PE never idles waiting for the Scalar engine's
   exp chain (which is the attention-phase bottleneck) and the HAM clock
   stays at full speed.
 - Scalar engine runs exp ONLY.  All evacuation copies moved to DVE/Pool.
 - PSUM: 2x2-bank score slots ("s"), 2x1-bank ctx slots ("c"), 2x1-bank
   filler slots ("f" - psv/psq-fillers/psy), so fillers never contend with
   the attention pipeline.
 - Q tiles are per-pair JOINT (both heads stacked in the 128 partitions);
   only the K tiles are per-head zero-padded, which the scores matmul needs
   on one side only.  Saves half the RoPE adds for q.
 - RoPE reads the QKV PSUM directly (shuffle + cos-mul from PSUM), no
   separate raw copy.
 - Diagonal score blocks extend the second half to full width so one exp
   instruction covers the whole [128,1024] tile (garbage cols never read).
 - V' layout puts odd heads' values at ctx partitions 64:128 with the
   denominator at partition 0 (even: values 0:64, denominator 64), so the
   softmax normalization multiply is partition-aligned for both heads.
 - Output in fp16 (half the store traffic); host sums partials in fp32.
"""

import math
import os
import sys

import numpy as np

sys.path.insert(0, "/opt/trn_rl_repo")

from contextlib import ExitStack

import concourse.bass as bass
import concourse.tile as tile
import concourse.mybir as mybir
from concourse import bacc, bass_utils

# Problem constants
B, T, HID = 2, 2048, 1024
H, D = 16, 64
NCORES = 8
KT = HID // 128  # 8 contraction tiles for qkv
F32 = mybir.dt.float32
BF16 = mybir.dt.bfloat16
F16 = mybir.dt.float16
SCALE = 1.0 / math.sqrt(D)
NPIPE = 3
# CoreSim tracks PSUM accumulation groups byte-precisely: every started byte
# must see a stop before it is read.  The wide masks / zero-closers that
# satisfy it add ~8.5us of real PE time, so they are only emitted for sim
# validation (numerics are identical: they only add zeros).
SIM_SAFE = bool(os.environ.get("BASS_SIM_SAFE"))

_NC_CACHE = {}


def build_nc():
    """Build + compile the per-core Bass program (identical SPMD on all cores)."""
    if "nc" in _NC_CACHE:
        return _NC_CACHE["nc"]

    nc = bacc.Bacc("TRN2", target_bir_lowering=False, debug=False)

    # DRAM I/O (per core)
    hx_d = nc.dram_tensor("hx", [128, 2 * KT * 1024], BF16, kind="ExternalInput").ap()
    wq_d = nc.dram_tensor("wq", [128, KT * 256], BF16, kind="ExternalInput").ap()
    wk_d = nc.dram_tensor("wk", [128, KT * 256], BF16, kind="ExternalInput").ap()
    wv_d = nc.dram_tensor("wv", [128, KT * 256], BF16, kind="ExternalInput").ap()
    cos_d = nc.dram_tensor("cosx", [128, T], BF16, kind="ExternalInput").ap()
    sin_d = nc.dram_tensor("sinx", [128, T], BF16, kind="ExternalInput").ap()
    wo_d = nc.dram_tensor("wo", [128, 2 * HID], BF16, kind="ExternalInput").ap()
    # [strict-upper -1e9 (lhsT) | I128 | zeros-384]: the identity+zeros slice
    # is the moving operand of the mask matmul, sized (in SIM_SAFE) to close
    # the whole scores accumulation region (adds 0 beyond the diagonal).
    mski_d = nc.dram_tensor("mski", [128, 640], BF16, kind="ExternalInput").ap()
    ones_d = nc.dram_tensor("ones", [128, 2048], BF16, kind="ExternalInput").ap()
    yt_d = nc.dram_tensor("yt", [128, 8 * T], F16, kind="ExternalOutput").ap()

    Exp = mybir.ActivationFunctionType.Exp
    SWAP_MASK = [i ^ 1 for i in range(32)]

    with tile.TileContext(nc) as tc:
        with ExitStack() as octx:
            big = octx.enter_context(tc.tile_pool(name="big", bufs=1))
            rp = octx.enter_context(tc.tile_pool(name="rp", bufs=1))
            ep = octx.enter_context(tc.tile_pool(name="ep", bufs=1))
            yp = octx.enter_context(tc.tile_pool(name="yp", bufs=1))
            psA = octx.enter_context(tc.tile_pool(name="psA", bufs=1, space="PSUM"))

            hx_t = big.tile([128, 2 * KT * 1024], BF16, name="hx_t")
            wq_t = big.tile([128, KT * 256], BF16, name="wq_t")
            wk_t = big.tile([128, KT * 256], BF16, name="wk_t")
            wv_t = big.tile([128, KT * 256], BF16, name="wv_t")
            cos_t = big.tile([128, T], BF16, name="cos_t")
            sin_t = big.tile([128, T], BF16, name="sin_t")
            wo_t = big.tile([128, 2 * HID], BF16, name="wo_t")
            mski_t = big.tile([128, 640], BF16, name="mski_t")
            # joint q pair tiles (head A rows 0:64, head B rows 64:128)
            QTP = [big.tile([128, T], BF16, name=f"QTP{p}") for p in range(2)]
            # per-head K tiles: even heads data 0:64 / zero 64:128, odd flipped
            KTt = [big.tile([128, T], BF16, name=f"KTt{h}") for h in range(4)]
            Vp = big.tile([128, 16 * 4 * 128], BF16, name="Vp")
            CTXU = [big.tile([128, T], BF16, name=f"CTXU{p}") for p in range(2)]

            # ---- input DMAs, ordered by first use.  hx halves split by x
            # (and the first chunk by k-group) so the first psq matmuls only
            # wait on ~0.75 MiB of transfer.
            def hx_x_ap(t, half, x, ks=slice(0, KT)):
                # cols {k*1024 + x*512 .. +512} for k in ks of this half
                return t[:, half * 8192:(half + 1) * 8192].rearrange(
                    "p (k x c) -> p k x c", k=KT, x=2
                )[:, ks, x, :]

            nc.scalar.dma_start(wq_t[:, 0:512], wq_d[:, 0:512])
            nc.scalar.dma_start(wk_t[:, 0:512], wk_d[:, 0:512])
            nc.sync.dma_start(hx_x_ap(hx_t, 0, 0, slice(0, 2)),
                              hx_x_ap(hx_d, 0, 0, slice(0, 2)))
            nc.sync.dma_start(hx_x_ap(hx_t, 0, 0, slice(2, 4)),
                              hx_x_ap(hx_d, 0, 0, slice(2, 4)))
            nc.sync.dma_start(wq_t[:, 512:1024], wq_d[:, 512:1024])
            nc.sync.dma_start(wk_t[:, 512:1024], wk_d[:, 512:1024])
            nc.sync.dma_start(hx_x_ap(hx_t, 0, 0, slice(4, KT)),
                              hx_x_ap(hx_d, 0, 0, slice(4, KT)))
            nc.sync.dma_start(wv_t[:], wv_d[:])
            nc.sync.dma_start(cos_t[:], cos_d[:])
            nc.sync.dma_start(sin_t[:], sin_d[:])
            nc.sync.dma_start(hx_x_ap(hx_t, 0, 1), hx_x_ap(hx_d, 0, 1))
            nc.sync.dma_start(mski_t[:], mski_d[:])
            nc.sync.dma_start(wq_t[:, 1024:2048], wq_d[:, 1024:2048])
            nc.sync.dma_start(wk_t[:, 1024:2048], wk_d[:, 1024:2048])
            nc.sync.dma_start(hx_x_ap(hx_t, 1, 0), hx_x_ap(hx_d, 1, 0))
            nc.sync.dma_start(hx_x_ap(hx_t, 1, 1), hx_x_ap(hx_d, 1, 1))
            nc.sync.dma_start(wo_t[:], wo_d[:])

            # zero pads (scalar is idle during the DMA ramp)
            nc.scalar.memzero(KTt[0][64:128, :])
            nc.scalar.memzero(KTt[1][0:64, :])
            nc.scalar.memzero(KTt[2][64:128, :])
            nc.scalar.memzero(KTt[3][0:64, :])
            # denominator ones BLOCKS: even heads cols 64:128, odd heads cols
            # 0:64.  The ctx matmul then lands 64 replicated denominator rows
            # in PSUM, so no cross-partition broadcast is ever needed.
            ones_src = ones_d[:, 0:2048].rearrange("p (s h2 c) -> p s h2 c",
                                                   s=16, h2=2)
            vp5 = Vp[:].rearrange("p (s h2 two c) -> p s h2 two c", s=16, h2=2, two=2)
            nc.sync.dma_start(vp5[:, :, :, 0, 64:128], ones_src)
            nc.sync.dma_start(vp5[:, :, :, 1, 0:64], ones_src)

            # ---------- QKV projection + RoPE for one 512-col chunk ----------
            def emit_psq(pr, which, half, x, ptag, fast=False, pair_with=None):
                w_t = wq_t if which == "q" else wk_t
                psq = psA.tile([128, 512], F32, name=f"psq{pr}{which}{half}{x}",
                               tag=ptag, bufs=2)
                psq2 = None
                if pair_with is not None:
                    which2, w_t2 = pair_with
                    psq2 = psA.tile([128, 512], F32,
                                    name=f"psq{pr}{which2}{half}{x}",
                                    tag=ptag, bufs=2)
                for k in range(KT):
                    hxs = hx_t[:, half * 8192 + k * 1024 + x * 512:
                               half * 8192 + k * 1024 + (x + 1) * 512]
                    nc.tensor.matmul(
                        psq[:],
                        w_t[:, pr * 1024 + k * 128: pr * 1024 + (k + 1) * 128],
                        hxs, start=(k == 0), stop=(k == KT - 1),
                    )
                    if psq2 is not None:
                        nc.tensor.matmul(
                            psq2[:],
                            w_t2[:, pr * 1024 + k * 128: pr * 1024 + (k + 1) * 128],
                            hxs, start=(k == 0), stop=(k == KT - 1),
                        )
                cs = slice(half * 1024 + x * 512, half * 1024 + (x + 1) * 512)
                pairs = [(which, psq)]
                if psq2 is not None:
                    pairs.append((pair_with[0], psq2))
                for which, psq in pairs:
                    emit_rope(pr, which, half, x, cs, psq, fast)

            def emit_rope(pr, which, half, x, cs, psq, fast):
                rot = rp.tile([128, 512], F32, name=f"rot{pr}{which}{half}{x}",
                              tag="rot", bufs=2)
                tmp1 = rp.tile([128, 512], BF16, name=f"t1{pr}{which}{half}{x}",
                               tag="tmp1", bufs=2)
                tmp2 = rp.tile([128, 512], BF16, name=f"t2{pr}{which}{half}{x}",
                               tag="tmp2", bufs=2)
                # Vector extracts from PSUM (shuffle + cos-mul); GpSimd does
                # the sin-mul and all adds so dependencies only flow V -> G
                # (no G -> V edges to head-of-line-block the Vector queue).
                # fast=True runs the whole chunk on Vector (low latency, for
                # the startup chunks that gate the first attention block).
                eng = nc.vector if fast else nc.gpsimd
                nc.vector.stream_shuffle(rot[:], psq[:], SWAP_MASK)
                nc.vector.tensor_mul(tmp1[:], psq[:], cos_t[:, cs])
                eng.tensor_mul(tmp2[:], rot[:], sin_t[:, cs])
                if which == "q":
                    eng.tensor_add(QTP[pr][:, cs], tmp1[:], tmp2[:])
                else:
                    eng.tensor_add(KTt[2 * pr][0:64, cs],
                                   tmp1[0:64, :], tmp2[0:64, :])
                    eng.tensor_add(KTt[2 * pr + 1][64:128, cs],
                                   tmp1[64:128, :], tmp2[64:128, :])

            # ---------- V' projection for one 128-row t-block ----------------
            def emit_v(st):
                half, lc = st // 8, st % 8
                psv = psA.tile([128, 256], F32, name=f"psv{st}", tag="f", bufs=2)
                for k in range(KT):
                    nc.tensor.matmul(
                        psv[:],
                        hx_t[:, half * 8192 + k * 1024 + lc * 128:
                             half * 8192 + k * 1024 + lc * 128 + 128],
                        wv_t[:, k * 256:(k + 1) * 256],
                        start=(k == 0), stop=(k == KT - 1),
                    )
                # single evac copy: even heads land at block cols 0:64, odd at
                # 64:128 (dst col = h2*256 + two*192 + c, an affine pattern)
                src = psv[:].rearrange("p (h2 two c) -> p h2 two c", h2=2, two=2)
                base = Vp[:, st * 512:(st + 1) * 512]
                dst = bass.AP(base.tensor, base.offset,
                              [list(base.ap[0]), [256, 2], [192, 2], [1, 64]])
                nc.vector.tensor_copy(dst, src)

            # ---------- output projection for one (n, m) block ---------------
            def emit_oproj(n, m, ptag="f", scalar_evac=False):
                psy = psA.tile([128, 512], F32, name=f"psy{n}{m}", tag=ptag, bufs=2)
                for jt in range(2):
                    nc.tensor.matmul(
                        psy[:],
                        wo_t[:, jt * HID + m * 128: jt * HID + (m + 1) * 128],
                        CTXU[jt][:, n * 512:(n + 1) * 512],
                        start=(jt == 0), stop=(jt == 1),
                    )
                yst = yp.tile([128, 512], F16, name=f"yst{n}{m}", tag="yst", bufs=6)
                if scalar_evac:
                    nc.scalar.copy(yst[:], psy[:])
                else:
                    nc.vector.tensor_copy(yst[:], psy[:])
                nc.sync.dma_start(
                    yt_d[:, m * 2048 + n * 512: m * 2048 + (n + 1) * 512], yst[:])

            # ---------- filler machinery -------------------------------------
            FILL = []

            def fill(n):
                for _ in range(n):
                    if FILL:
                        FILL.pop(0)()

            # ---------- attention for one head pair, one 512-query block -----
            def emit_attn_tb(pr, tb, fills):
                hA, hB = 2 * pr, 2 * pr + 1
                shi = 4 * (tb + 1)
                ctxP = [
                    psA.tile([128, 512], F32, name=f"ctx{pr}{tb}{ab}", tag="c",
                             bufs=2)
                    for ab in range(2)
                ]
                pend = []

                def flush_ctx(p):
                    si, col0, c0, c1, ees = p
                    for ab, hh in ((0, hA), (1, hB)):
                        off = (si * 4 + hh) * 128
                        nc.tensor.matmul(
                            ctxP[ab][:, col0:],
                            Vp[:, off: off + 128],
                            ees[ab][:, c0:c1],
                            start=(si == 0), stop=(not SIM_SAFE and si == shi - 1),
                        )
                        if SIM_SAFE and si == shi - 1:
                            # zero-weight closer: adds 0, carries the group
                            # stop across the full ctx region (the diagonal
                            # si blocks only cover a shrinking col range)
                            nc.tensor.matmul(
                                ctxP[ab][:, :],
                                mski_t[:, 512:640],
                                mski_t[:, 0:512],
                                start=False, stop=True,
                            )

                for sp in range(shi // 2):
                    sis = (2 * sp, 2 * sp + 1)
                    j0 = sis[0] - 4 * tb
                    diag = j0 >= 0
                    ees = []
                    for ab, hh in ((0, hA), (1, hB)):
                        pss = psA.tile([128, 1024], F32, name=f"s{pr}{tb}{sp}{ab}",
                                       tag="s", bufs=2)
                        for x, si in enumerate(sis):
                            j = si - 4 * tb
                            col0 = 128 * j if j > 0 else 0
                            nc.tensor.matmul(
                                pss[:, x * 512 + col0:(x + 1) * 512],
                                KTt[hh][:, si * 128:(si + 1) * 128],
                                QTP[pr][:, tb * 512 + col0:(tb + 1) * 512],
                                start=True, stop=(j < 0),
                            )
                        if diag:
                            # -1e9 causal mask on the diagonal 128 block (on
                            # the PE: it is off every other engine's critical
                            # path).  In SIM_SAFE the mask widens with zeros so
                            # its group stop covers every started byte.
                            for x, si in enumerate(sis):
                                j = si - 4 * tb
                                w = (512 - 128 * j) if SIM_SAFE else 128
                                c = x * 512 + 128 * j
                                nc.tensor.matmul(
                                    pss[:, c:c + w],
                                    mski_t[:, 0:128],
                                    mski_t[:, 128:128 + w],
                                    start=False, stop=True,
                                    skip_group_check=not SIM_SAFE,
                                )
                        ee = ep.tile([128, 1024], BF16, name=f"e{pr}{tb}{sp}{ab}",
                                     tag="e", bufs=6)
                        if diag:
                            # two exps, each starting as soon as its half's
                            # mask lands (also skips the never-written gap)
                            for x, si in enumerate(sis):
                                c = x * 512 + 128 * (si - 4 * tb)
                                nc.scalar.activation(ee[:, c:(x + 1) * 512],
                                                     pss[:, c:(x + 1) * 512],
                                                     Exp, scale=SCALE)
                        else:
                            nc.scalar.activation(ee[:], pss[:], Exp, scale=SCALE)
                        ees.append(ee)
                    for x, si in enumerate(sis):
                        j = si - 4 * tb
                        col0 = 0 if j < 0 else 128 * j
                        pend.append((si, col0, x * 512 + col0, (x + 1) * 512, ees))
                        if len(pend) > NPIPE:
                            flush_ctx(pend.pop(0))
                    fill(fills[sp] if sp < len(fills) else 0)
                while pend:
                    flush_ctx(pend.pop(0))

                # softmax normalization.  ctxP already holds 64 replicated
                # denominator rows (even head: rows 64:128, odd: rows 0:64),
                # so the whole chain runs on the Vector engine: shifted copy
                # out of PSUM, reciprocal, then the normalize multiply.
                for ab in range(2):
                    rb = ep.tile([128, 512], F32, name=f"rb{pr}{tb}{ab}", tag="rb",
                                 bufs=3)
                    dsl = slice(64, 128) if ab == 0 else slice(0, 64)
                    sl = slice(0, 64) if ab == 0 else slice(64, 128)
                    nc.vector.tensor_copy(rb[sl, :], ctxP[ab][dsl, :])
                    if SIM_SAFE:
                        # the full-tile reciprocal reads the other 64 rows
                        # too; they are dead values on HW, but CoreSim flags
                        # uninitialized reads, so fill them for validation
                        nc.vector.tensor_copy(rb[dsl, :], ctxP[ab][dsl, :])
                    nc.vector.reciprocal_approx_fast(rb[:, :], rb[:, :])
                    nc.vector.tensor_mul(CTXU[pr][sl, tb * 512:(tb + 1) * 512],
                                         ctxP[ab][sl, :], rb[sl, :])

            # =================== schedule ====================================
            # upfront: just enough for attn(p0, tb0).  q and k chains are
            # interleaved per k-tile so each arriving hx DMA chunk feeds two
            # matmuls (halves the exposed chunk latency).
            emit_psq(0, "q", 0, 0, "s", fast=True, pair_with=("k", wk_t))
            for st in range(4):
                emit_v(st)

            FILL.extend([
                # consumed during attn(p0, tb0..tb3)
                lambda: emit_psq(0, "q", 0, 1, "f", fast=True),
                lambda: emit_psq(0, "k", 0, 1, "f", fast=True),
                lambda: emit_v(4), lambda: emit_v(5),
                lambda: emit_v(6), lambda: emit_v(7),
                lambda: emit_psq(0, "q", 1, 0, "f"),
                lambda: emit_psq(0, "q", 1, 1, "f"),
                lambda: emit_psq(0, "k", 1, 0, "f"),
                lambda: emit_psq(0, "k", 1, 1, "f"),
                lambda: emit_v(8), lambda: emit_v(9),
                lambda: emit_v(10), lambda: emit_v(11),
                lambda: emit_v(12), lambda: emit_v(13),
                lambda: emit_v(14), lambda: emit_v(15),
                lambda: emit_psq(1, "q", 0, 0, "f"),
                lambda: emit_psq(1, "k", 0, 0, "f"),
                lambda: emit_psq(1, "q", 0, 1, "f"),
                lambda: emit_psq(1, "k", 0, 1, "f"),
                lambda: emit_psq(1, "q", 1, 0, "f"),
                lambda: emit_psq(1, "k", 1, 0, "f"),
            ])

            emit_attn_tb(0, 0, [1, 1])
            emit_attn_tb(0, 1, [2, 2, 2, 2])
            emit_attn_tb(0, 2, [2, 2, 2, 2, 2, 2])
            emit_attn_tb(0, 3, [1, 1, 1, 1, 1, 1, 1, 1])
            fill(len(FILL))  # drain any leftovers before pair-1 attention

            # pair-1 t-blocks ordered 1,2,3,0: the LAST normalization chain
            # (which the final oproj blocks wait on) then belongs to the
            # smallest t-block, and the tail shrinks to ~12 filler units.
            FILL.extend([
                lambda: emit_psq(1, "q", 1, 1, "f"),
                lambda: emit_psq(1, "k", 1, 1, "f"),
            ])
            emit_attn_tb(1, 1, [1, 1, 0, 0])
            FILL.extend([lambda m=m: emit_oproj(1, m) for m in range(8)])
            emit_attn_tb(1, 2, [1, 1, 1, 1, 1, 1])
            FILL.extend([lambda m=m: emit_oproj(2, m) for m in range(8)])
            emit_attn_tb(1, 3, [2, 1, 1, 1, 1, 1, 1, 2])
            FILL.extend([lambda m=m: emit_oproj(3, m) for m in range(4)])
            emit_attn_tb(1, 0, [2, 2])
            # tail: cycle psy through all three PSUM tags (scores/ctx banks
            # are free now) and alternate evacs between Scalar (idle after
            # the last exp) and Vector, so the drain pipeline is deep.
            tail = [(3, m) for m in range(4, 8)] + [(0, m) for m in range(8)]
            for i, (n, m) in enumerate(tail):
                emit_oproj(n, m, ptag=("f", "s", "c")[i % 3],
                           scalar_evac=(i % 2 == 1))

    nc.compile()
    _NC_CACHE["nc"] = nc
    return nc


# RoPE head-dim permutation: d' = 2i -> i, 2i+1 -> 32+i
_PERM = np.empty(64, dtype=np.int64)
_PERM[0::2] = np.arange(32)
_PERM[1::2] = np.arange(32, 64)

_BF16 = mybir.dt.np(BF16)


def _mski() -> np.ndarray:
    maskT = np.triu(np.full((128, 128), -1e9, dtype=np.float32), 1)
    ident = np.eye(128, dtype=np.float32)
    zer = np.zeros((128, 384), dtype=np.float32)
    return np.concatenate([maskT, ident, zer], 1).astype(_BF16)


def _ktile_pack(a_t: np.ndarray) -> np.ndarray:
    """[HID, F] -> [128, KT*F] with k-tile-major free layout."""
    f = a_t.shape[1]
    return np.ascontiguousarray(
        a_t.reshape(KT, 128, f).transpose(1, 0, 2).reshape(128, KT * f)
    )


def _hx_pack(x_t: np.ndarray) -> np.ndarray:
    """[HID, T] -> [128, 2*KT*1024]: half-major, then k-tile, then col."""
    return np.ascontiguousarray(
        x_t.reshape(KT, 128, 2, 1024).transpose(1, 2, 0, 3).reshape(128, 2 * KT * 1024)
    )


def _w_pack(a_t: np.ndarray) -> np.ndarray:
    """[HID, 256] -> [128, 2*KT*128]: pair-major, then k-tile, then col."""
    return np.ascontiguousarray(
        a_t.reshape(KT, 128, 2, 128).transpose(1, 2, 0, 3).reshape(128, 2 * KT * 128)
    )


def prep_inputs(hidden_states, cos, sin, w_qkv, w_o):
    """Build the 8 per-core input maps."""
    hidden_states = np.asarray(hidden_states, dtype=np.float32)
    cos = np.asarray(cos, dtype=np.float32)
    sin = np.asarray(sin, dtype=np.float32)
    w_qkv = np.asarray(w_qkv, dtype=np.float32)
    w_o = np.asarray(w_o, dtype=np.float32)

    sgn = np.empty((64, 1), dtype=np.float32)
    sgn[0::2] = -1.0
    sgn[1::2] = 1.0
    cosx_half = cos.T[_PERM]                 # [64, T]
    sinx_half = sin.T[_PERM] * sgn           # [64, T]
    cosx = np.ascontiguousarray(np.concatenate([cosx_half, cosx_half], 0)).astype(_BF16)
    sinx = np.ascontiguousarray(np.concatenate([sinx_half, sinx_half], 0)).astype(_BF16)

    in_maps = []
    for c in range(NCORES):
        b, g = c // 4, c % 4
        r0 = g * 256
        wq_rows = w_qkv[r0: r0 + 256].reshape(4, 64, HID)[:, _PERM, :].reshape(256, HID)
        wk_rows = w_qkv[HID + r0: HID + r0 + 256].reshape(4, 64, HID)[:, _PERM, :]
        wk_rows = wk_rows.reshape(256, HID)
        wv_rows = w_qkv[2 * HID + r0: 2 * HID + r0 + 256]
        wo_cols = w_o[:, r0: r0 + 256]       # [HID, 256]

        in_maps.append({
            "hx": _hx_pack(hidden_states[b].T).astype(_BF16),
            "wq": _w_pack(wq_rows.T).astype(_BF16),
            "wk": _w_pack(wk_rows.T).astype(_BF16),
            "wv": _ktile_pack(wv_rows.T).astype(_BF16),
            "wo": np.ascontiguousarray(
                wo_cols.T.reshape(2, 128, HID).transpose(1, 0, 2).reshape(128, 2 * HID)
            ).astype(_BF16),
            "cosx": cosx,
            "sinx": sinx,
            "ones": np.ones((128, 2048), dtype=_BF16),
            "mski": _mski(),
        })
    return in_maps


def assemble_output(results):
    """Sum the 4 per-batch fp16 partials and transpose back to [B, T, HID]."""
    out = np.zeros((B, T, HID), dtype=np.float32)
    for c in range(NCORES):
        b = c // 4
        yt = results[c]["yt"].astype(np.float32)          # [128, 8*T]
        ytf = yt.reshape(128, 8, T).transpose(1, 0, 2).reshape(HID, T)
        out[b] += ytf.T
    return out


def run(inputs: dict, trace: bool = False, tmpdir: str | None = None):
    nc = build_nc()
    in_maps = prep_inputs(**inputs)
    res = bass_utils.run_bass_kernel_spmd(
        nc, in_maps, core_ids=list(range(NCORES)), trace=trace, tmpdir=tmpdir
    )
    return assemble_output(res.results), res


def kernel(**inputs) -> np.ndarray:
    out, _ = run(inputs, trace=False)
    return out
